# revision 34
# baseline (speedup 1.0000x reference)
"""Trainium2 Bass kernel for linear attention (silu+1 feature map, cumsum over T)
with dense 1024x1024 in/out projections.

Sharding: 8 cores = 4 batches x 2 head-groups (8 heads / 512 channels each).
Each core computes q/k/v projections for its 512 channels over the full
T=4096 of its batch, the linear-attention recurrence locally (DVE prefix
scan along T), and a partial Wo projection (512 in-ch -> all 1024 out-ch).
The host sums the two partials per batch and adds bo. No cross-core traffic.

Layout: channels on partitions, tokens on the free dim. All matmuls stream
bf16 operands into fp32 PSUM. The cumulative sums, feature maps and the
denominator/reciprocal run in fp32.
"""

import numpy as np
import ml_dtypes

import concourse.bass as bass
import concourse.mybir as mybir
from concourse import bacc, tile
from concourse.bass_utils import run_bass_kernel_spmd

BF16 = mybir.dt.bfloat16
F32 = mybir.dt.float32
FP8 = mybir.dt.float8e4
DR = mybir.MatmulPerfMode.DoubleRow
XS = 0.125        # host scales x by XS, weights by 1/(XS*PS)
PS = 0.125        # ACT scale undoing the fp8 pre-scaling: psum*PS = true value
ADD = mybir.AluOpType.add
MULT = mybir.AluOpType.mult
BYPASS = mybir.AluOpType.bypass
SILU = mybir.ActivationFunctionType.Silu
COPY = mybir.ActivationFunctionType.Copy
IDENT = mybir.ActivationFunctionType.Identity

B, C, T = 4, 1024, 4096
H, DH = 16, 64
CG = 512            # channels per head-group (per core)
S = 512             # token slab
NSLAB = T // S      # 8
NCH = CG // 128     # 4 chunks of 128 channels
KCH = C // 128      # 8 input-channel chunks
MO = C // 128       # 8 output-channel chunks


def build():
    """Build the per-core Bass program (identical on all 8 cores)."""
    nc = bacc.Bacc(target_bir_lowering=False)

    x_d = nc.declare_dram_parameter("x", [C, T], FP8, isOutput=False)
    wq_d = nc.declare_dram_parameter("wq", [C, CG], FP8, isOutput=False)
    wk_d = nc.declare_dram_parameter("wk", [C, CG], FP8, isOutput=False)
    wv_d = nc.declare_dram_parameter("wv", [C, CG], FP8, isOutput=False)
    wo_d = nc.declare_dram_parameter("wo", [CG, C], FP8, isOutput=False)
    bq_d = nc.declare_dram_parameter("bq", [CG, 1], F32, isOutput=False)
    bv_d = nc.declare_dram_parameter("bv", [CG, 1], F32, isOutput=False)
    em_d = nc.declare_dram_parameter("emat", [CG, 8], BF16, isOutput=False)
    fm_d = nc.declare_dram_parameter("fmat", [8, CG], BF16, isOutput=False)
    on_d = nc.declare_dram_parameter("ones", [128, S], BF16, isOutput=False)
    out_d = nc.declare_dram_parameter("out", [C, T], F32, isOutput=True)

    with tile.TileContext(nc) as tc:
        from contextlib import ExitStack

        with ExitStack() as ctx:
            wpool = ctx.enter_context(tc.tile_pool(name="w", bufs=1))
            xpool = ctx.enter_context(tc.tile_pool(name="xp", bufs=3))
            ppool = ctx.enter_context(tc.tile_pool(name="proj", bufs=3, space="PSUM"))
            dpool = ctx.enter_context(tc.tile_pool(name="denp", bufs=1, space="PSUM"))
            bpool = ctx.enter_context(tc.tile_pool(name="bcast", bufs=2, space="PSUM"))
            opool = ctx.enter_context(tc.tile_pool(name="wops", bufs=2, space="PSUM"))
            apool = ctx.enter_context(tc.tile_pool(name="act", bufs=3))
            spool = ctx.enter_context(tc.tile_pool(name="state", bufs=3))
            outpool = ctx.enter_context(tc.tile_pool(name="outp", bufs=2))

            # ---- persistent weights/constants in SBUF
            def load(pool, shape, dtype, src, tag):
                t = pool.tile(shape, dtype, tag=tag, name=tag)
                nc.scalar.dma_start(t[:], src)
                return t

            wq_t = wk_t = wv_t = wo_t = None  # loaded after slab-0 x DMAs
            em_t = fm_t = bq_t = bv_t = ones_t = None

            prev_ks = [None] * NCH
            prev_kvs = [None] * NCH
            prev_tail = None
            prev_tail2 = None

            SLABS = [(512 * s, 512) for s in range(NSLAB)]
            prev_len = None
            for s, (t0, SL) in enumerate(SLABS):
                ts = slice(t0, t0 + SL)
                x_t = []
                for k in range(KCH // 2):
                    xt = xpool.tile([128, 2, SL], FP8, tag=f"x{k}", name=f"x{k}_{s}")
                    nc.sync.dma_start(xt[:], x_d[256 * k : 256 * (k + 1), ts].rearrange("(ko ki) t -> ki ko t", ko=2))
                    x_t.append(xt)
                if wq_t is None:
                    def load_w8(dram, tagp, n, fd):
                        tiles = []
                        for k in range(n):
                            t = wpool.tile([128, 2, fd], FP8, tag=f"{tagp}{k}", name=f"{tagp}{k}")
                            nc.scalar.dma_start(t[:], dram[256 * k : 256 * (k + 1), :].rearrange("(ko ki) m -> ki ko m", ko=2))
                            tiles.append(t)
                        return tiles
                    wq_t = load_w8(wq_d, "wq", KCH // 2, CG)
                    wk_t = load_w8(wk_d, "wk", KCH // 2, CG)
                    wv_t = load_w8(wv_d, "wv", KCH // 2, CG)
                    bq_t = [load(wpool, [128, 1], F32, bq_d[128 * c : 128 * (c + 1), :], f"bq{c}") for c in range(NCH)]
                    bv_t = [load(wpool, [128, 1], F32, bv_d[128 * c : 128 * (c + 1), :], f"bv{c}") for c in range(NCH)]
                    ones_t = load(wpool, [128, S], BF16, on_d[:, :], "ones")
                    em_t = [load(wpool, [128, 8], BF16, em_d[128 * c : 128 * (c + 1), :], f"em{c}") for c in range(NCH)]
                    fm_t = load(wpool, [8, CG], BF16, fm_d[:, :], "fm")
                    wo_t = load_w8(wo_d, "wo", NCH // 2, C)

                sq_l, ks_l, kvs_l, pq_l = [], [], [], []
                for c in range(NCH):
                    cs = slice(128 * c, 128 * (c + 1))
                    # projections: psum[ch_out 128, tok 512] += wT_chunk.T @ x_chunk
                    K2 = KCH // 2
                    ps_q = ppool.tile([128, SL], F32, tag="proj", name=f"psq{s}_{c}")
                    for k in range(K2):
                        nc.tensor.matmul(ps_q[:], wq_t[k][:, :, cs], x_t[k][:], start=(k == 0), stop=(k == K2 - 1), perf_mode=DR)
                    ps_k = ppool.tile([128, SL], F32, tag="proj", name=f"psk{s}_{c}")
                    for k in range(K2):
                        nc.tensor.matmul(ps_k[:], wk_t[k][:, :, cs], x_t[k][:], start=(k == 0), stop=(k == K2 - 1), perf_mode=DR)
                    ps_v = ppool.tile([128, SL], F32, tag="proj", name=f"psv{s}_{c}")
                    for k in range(K2):
                        nc.tensor.matmul(ps_v[:], wv_t[k][:, :, cs], x_t[k][:], start=(k == 0), stop=(k == K2 - 1), perf_mode=DR)

                    sq = apool.tile([128, SL], BF16, tag=f"sq{c}", name=f"sq{s}_{c}")
                    nc.scalar.activation(sq[:], ps_q[:], SILU, bias=bq_t[c][:], scale=PS)
                    sk = apool.tile([128, SL], BF16, tag=f"sk{c}", name=f"sk{s}_{c}")
                    nc.scalar.activation(sk[:], ps_k[:], SILU, scale=PS)
                    vs = apool.tile([128, SL], BF16, tag=f"vs{c}", name=f"vs{s}_{c}")
                    nc.scalar.activation(vs[:], ps_v[:], IDENT, bias=bv_t[c][:], scale=PS)

                    # pkv = (silu_k + 1) * v
                    pk = apool.tile([128, SL], BF16, tag=f"pkv{c}", name=f"pkv{s}_{c}")
                    nc.vector.scalar_tensor_tensor(pk[:], sk[:], 1.0, vs[:], op0=ADD, op1=MULT)

                    # k_sum = cumsum(silu_k + 1), kv_sum = cumsum(pkv), chained across slabs
                    ks = spool.tile([128, SL], BF16, tag=f"ks{c}", name=f"ks{s}_{c}")
                    ik = 0.0 if s == 0 else prev_ks[c][:, prev_len - 1 : prev_len]
                    nc.vector.tensor_tensor_scan(ks[:], sk[:], ones_t[:, :SL], initial=ik, op0=ADD, op1=ADD)
                    kvs = spool.tile([128, SL], BF16, tag=f"kvs{c}", name=f"kvs{s}_{c}")
                    ikv = 0.0 if s == 0 else prev_kvs[c][:, prev_len - 1 : prev_len]
                    nc.vector.tensor_tensor_scan(kvs[:], pk[:], ones_t[:, :SL], initial=ikv, op0=ADD, op1=BYPASS)
                    prev_ks[c], prev_kvs[c] = ks, kvs

                    # phi_q = silu_q + 1 (4x tensor_scalar), pq_ks = phi_q * k_sum (2x TT)
                    phq = apool.tile([128, SL], BF16, tag=f"phq{c}", name=f"phq{s}_{c}")
                    nc.vector.tensor_scalar_add(phq[:], sq[:], 1.0)
                    pq = apool.tile([128, SL], BF16, tag=f"pq{c}", name=f"pq{s}_{c}")
                    nc.vector.tensor_mul(pq[:], phq[:], ks[:])

                    sq_l.append(phq), ks_l.append(ks), kvs_l.append(kvs), pq_l.append(pq)

                def tail_a(s, tt0, TL, sq_l, kvs_l, pq_l):
                    """den -> rec for slab s (emitted 1 slab late)."""
                    den_ps = dpool.tile([8, TL], F32, tag="den", name=f"den{s}")
                    for c in range(NCH):
                        nc.tensor.matmul(den_ps[:], em_t[c][:], pq_l[c][:], start=(c == 0), stop=(c == NCH - 1))
                    rec32 = apool.tile([8, TL], F32, tag="rec32", name=f"rec32{s}")
                    nc.vector.reciprocal_approx_fast(rec32[:], den_ps[:])
                    rec = apool.tile([8, TL], BF16, tag="rec", name=f"rec{s}")
                    nc.scalar.copy(rec[:], rec32[:])
                    return rec

                def tail_b(s, tt0, TL, sq_l, kvs_l, pq_l, rec):
                    """broadcast -> attn -> Wo for slab s (emitted 2 slabs late,
                    so rec is long since ready when the PE reaches the bcast)."""
                    tts = slice(tt0, tt0 + TL)
                    at_l = [apool.tile([128, 2, TL], FP8, tag=f"at{cc}", name=f"at{s}_{cc}") for cc in range(NCH // 2)]
                    for c in range(NCH):
                        cs = slice(128 * c, 128 * (c + 1))
                        rb = bpool.tile([128, TL], F32, tag="rb", name=f"rb{s}_{c}")
                        nc.tensor.matmul(rb[:], fm_t[:, cs], rec[:], start=True, stop=True)
                        nm = apool.tile([128, TL], BF16, tag=f"nm{c}", name=f"nm{s}_{c}")
                        nc.vector.tensor_mul(nm[:], sq_l[c][:], kvs_l[c][:])
                        nc.vector.tensor_mul(at_l[c // 2][:, c % 2, :], nm[:], rb[:])

                    for moo in range(MO // 2):
                        ot = outpool.tile([128, 2, TL], F32, tag=f"ot{moo}", name=f"ot{s}_{moo}")
                        for mo2 in range(2):
                            mo = 2 * moo + mo2
                            ms = slice(128 * mo, 128 * (mo + 1))
                            wo_ps = opool.tile([128, TL], F32, tag="wo", name=f"wo{s}_{mo}")
                            for kk in range(NCH // 2):
                                nc.tensor.matmul(wo_ps[:], wo_t[kk][:, :, ms], at_l[kk][:], start=(kk == 0), stop=(kk == NCH // 2 - 1), perf_mode=DR)
                            nc.scalar.activation(ot[:, mo2, :], wo_ps[:], COPY, scale=1.0 / 64.0)
                        nc.gpsimd.dma_start(
                            out_d[256 * moo : 256 * (moo + 1), tts].rearrange("(mo2 ki) t -> ki mo2 t", mo2=2),
                            ot[:])

                if prev_tail is not None:
                    rec_prev = tail_a(*prev_tail)
                    if prev_tail2 is not None:
                        tail_b(*prev_tail2)
                    prev_tail2 = (*prev_tail, rec_prev)
                prev_tail = (s, t0, SL, sq_l, kvs_l, pq_l)
                prev_len = SL

            rec_prev = tail_a(*prev_tail)
            tail_b(*prev_tail2)
            tail_b(*prev_tail, rec_prev)

    nc.compile()
    return nc


_NC_CACHE = {}


def _get_nc():
    if "nc" not in _NC_CACHE:
        _NC_CACHE["nc"] = build()
    return _NC_CACHE["nc"]


def make_in_maps(x, Wq, bq, Wk, Wv, bv, Wo, bo):
    bf = ml_dtypes.bfloat16
    f8 = ml_dtypes.float8_e4m3
    WS = 1.0 / (XS * PS)  # weight pre-scale so that psum * PS = W @ x exactly
    x3 = np.asarray(x, np.float32)[..., 0]                      # (B, C, T)
    E = np.zeros((CG, 8), np.float32)
    for ch in range(CG):
        E[ch, ch // DH] = 1.0
    ones = np.ones((128, S), bf)
    in_maps = []
    for core in range(8):
        b, g = core // 2, core % 2
        sl = slice(CG * g, CG * (g + 1))
        in_maps.append({
            "x": np.clip(x3[b] * XS, -240, 240).astype(f8),
            "wq": np.clip(np.ascontiguousarray(np.asarray(Wq, np.float32)[sl, :].T) * WS, -240, 240).astype(f8),
            "wk": np.clip(np.ascontiguousarray(np.asarray(Wk, np.float32)[sl, :].T) * WS, -240, 240).astype(f8),
            "wv": np.clip(np.ascontiguousarray(np.asarray(Wv, np.float32)[sl, :].T) * WS, -240, 240).astype(f8),
            "wo": np.clip(np.ascontiguousarray((np.asarray(Wo, np.float32)[:, sl] * 0.125).T) * 64.0, -240, 240).astype(f8),
            "bq": np.asarray(bq, np.float32)[sl].reshape(CG, 1).copy(),
            "bv": np.asarray(bv, np.float32)[sl].reshape(CG, 1).copy(),
            "emat": E.astype(bf),
            "fmat": np.ascontiguousarray(E.T).astype(bf),
            "ones": ones,
        })
    return in_maps


def assemble(results, bo):
    out = np.empty((B, C, T, 1), np.float32)
    bo_f = np.asarray(bo, np.float32)[:, None]
    for b in range(B):
        p0 = np.asarray(results[2 * b]["out"], np.float32)
        p1 = np.asarray(results[2 * b + 1]["out"], np.float32)
        out[b, :, :, 0] = p0 + p1 + bo_f
    return out


def kernel(x, Wq, bq, Wk, Wv, bv, Wo, bo):
    nc = _get_nc()
    in_maps = make_in_maps(x, Wq, bq, Wk, Wv, bv, Wo, bo)
    res = run_bass_kernel_spmd(nc, in_maps, core_ids=list(range(8)))
    return assemble(res.results, bo)


# revision 35
# speedup vs baseline: 1.2468x; 1.2468x over previous
"""Trainium2 Bass kernel for linear attention (silu+1 feature map, cumsum over T)
with dense 1024x1024 in/out projections.

Sharding: 8 cores = 4 batches x 2 head-groups (8 heads / 512 channels each).
Each core computes q/k/v projections for its 512 channels over the full
T=4096 of its batch, the linear-attention recurrence locally (DVE prefix
scan along T), and a partial Wo projection (512 in-ch -> all 1024 out-ch).
The host sums the two partials per batch and adds bo. No cross-core traffic.

Layout: channels on partitions, tokens on the free dim. All matmuls stream
bf16 operands into fp32 PSUM. The cumulative sums, feature maps and the
denominator/reciprocal run in fp32.
"""

import numpy as np
import ml_dtypes

import concourse.bass as bass
import concourse.mybir as mybir
from concourse import bacc, tile
from concourse.bass_utils import run_bass_kernel_spmd

BF16 = mybir.dt.bfloat16
F32 = mybir.dt.float32
FP8 = mybir.dt.float8e4
DR = mybir.MatmulPerfMode.DoubleRow
XS = 0.125        # host scales x by XS, weights by 1/(XS*PS)
PS = 0.125        # ACT scale undoing the fp8 pre-scaling: psum*PS = true value
ADD = mybir.AluOpType.add
MULT = mybir.AluOpType.mult
BYPASS = mybir.AluOpType.bypass
SILU = mybir.ActivationFunctionType.Silu
COPY = mybir.ActivationFunctionType.Copy
IDENT = mybir.ActivationFunctionType.Identity

B, C, T = 4, 1024, 4096
H, DH = 16, 64
CG = 512            # channels per head-group (per core)
S = 512             # token slab
NSLAB = T // S      # 8
NCH = CG // 128     # 4 chunks of 128 channels
KCH = C // 128      # 8 input-channel chunks
MO = C // 128       # 8 output-channel chunks


def build():
    """Build the per-core Bass program (identical on all 8 cores)."""
    nc = bacc.Bacc(target_bir_lowering=False)

    x_d = nc.declare_dram_parameter("x", [C, T], FP8, isOutput=False)
    wq_d = nc.declare_dram_parameter("wq", [C, CG], FP8, isOutput=False)
    wk_d = nc.declare_dram_parameter("wk", [C, CG], FP8, isOutput=False)
    wv_d = nc.declare_dram_parameter("wv", [C, CG], FP8, isOutput=False)
    wo_d = nc.declare_dram_parameter("wo", [CG, C], FP8, isOutput=False)
    bq_d = nc.declare_dram_parameter("bq", [CG, 1], F32, isOutput=False)
    bv_d = nc.declare_dram_parameter("bv", [CG, 1], F32, isOutput=False)
    em_d = nc.declare_dram_parameter("emat", [CG, 8], BF16, isOutput=False)
    fm_d = nc.declare_dram_parameter("fmat", [8, CG], BF16, isOutput=False)
    on_d = nc.declare_dram_parameter("ones", [128, S], BF16, isOutput=False)
    out_d = nc.declare_dram_parameter("out", [C, T], F32, isOutput=True)

    with tile.TileContext(nc) as tc:
        from contextlib import ExitStack

        with ExitStack() as ctx:
            wpool = ctx.enter_context(tc.tile_pool(name="w", bufs=1))
            xpool = ctx.enter_context(tc.tile_pool(name="xp", bufs=3))
            ppool = ctx.enter_context(tc.tile_pool(name="proj", bufs=3, space="PSUM"))
            dpool = ctx.enter_context(tc.tile_pool(name="denp", bufs=1, space="PSUM"))
            bpool = ctx.enter_context(tc.tile_pool(name="bcast", bufs=2, space="PSUM"))
            opool = ctx.enter_context(tc.tile_pool(name="wops", bufs=2, space="PSUM"))
            apool = ctx.enter_context(tc.tile_pool(name="act", bufs=3))
            spool = ctx.enter_context(tc.tile_pool(name="state", bufs=3))
            outpool = ctx.enter_context(tc.tile_pool(name="outp", bufs=2))

            # ---- persistent weights/constants in SBUF
            def load(pool, shape, dtype, src, tag):
                t = pool.tile(shape, dtype, tag=tag, name=tag)
                nc.sync.dma_start(t[:], src)
                return t

            wq_t = wk_t = wv_t = wo_t = None  # loaded after slab-0 x DMAs
            em_t = fm_t = bq_t = bv_t = ones_t = None

            prev_ks = [None] * NCH
            prev_kvs = [None] * NCH
            prev_tail = None
            prev_tail2 = None

            SLABS = [(512 * s, 512) for s in range(NSLAB)]
            prev_len = None
            for s, (t0, SL) in enumerate(SLABS):
                ts = slice(t0, t0 + SL)
                x_t = []
                for k in range(KCH // 2):
                    xt = xpool.tile([128, 2, SL], FP8, tag=f"x{k}", name=f"x{k}_{s}")
                    nc.sync.dma_start(xt[:], x_d[256 * k : 256 * (k + 1), ts].rearrange("(ko ki) t -> ki ko t", ko=2))
                    x_t.append(xt)
                if wq_t is None:
                    def load_w8(dram, tagp, n, fd):
                        tiles = []
                        for k in range(n):
                            t = wpool.tile([128, 2, fd], FP8, tag=f"{tagp}{k}", name=f"{tagp}{k}")
                            nc.sync.dma_start(t[:], dram[256 * k : 256 * (k + 1), :].rearrange("(ko ki) m -> ki ko m", ko=2))
                            tiles.append(t)
                        return tiles
                    wq_t = load_w8(wq_d, "wq", KCH // 2, CG)
                    wk_t = load_w8(wk_d, "wk", KCH // 2, CG)
                    wv_t = load_w8(wv_d, "wv", KCH // 2, CG)
                    bq_t = [load(wpool, [128, 1], F32, bq_d[128 * c : 128 * (c + 1), :], f"bq{c}") for c in range(NCH)]
                    bv_t = [load(wpool, [128, 1], F32, bv_d[128 * c : 128 * (c + 1), :], f"bv{c}") for c in range(NCH)]
                    ones_t = load(wpool, [128, S], BF16, on_d[:, :], "ones")
                    em_t = [load(wpool, [128, 8], BF16, em_d[128 * c : 128 * (c + 1), :], f"em{c}") for c in range(NCH)]
                    fm_t = load(wpool, [8, CG], BF16, fm_d[:, :], "fm")
                    wo_t = load_w8(wo_d, "wo", NCH // 2, C)

                sq_l, ks_l, kvs_l, pq_l = [], [], [], []
                for c in range(NCH):
                    cs = slice(128 * c, 128 * (c + 1))
                    # projections: psum[ch_out 128, tok 512] += wT_chunk.T @ x_chunk
                    K2 = KCH // 2
                    ps_q = ppool.tile([128, SL], F32, tag="proj", name=f"psq{s}_{c}")
                    for k in range(K2):
                        nc.tensor.matmul(ps_q[:], wq_t[k][:, :, cs], x_t[k][:], start=(k == 0), stop=(k == K2 - 1), perf_mode=DR)
                    ps_k = ppool.tile([128, SL], F32, tag="proj", name=f"psk{s}_{c}")
                    for k in range(K2):
                        nc.tensor.matmul(ps_k[:], wk_t[k][:, :, cs], x_t[k][:], start=(k == 0), stop=(k == K2 - 1), perf_mode=DR)
                    ps_v = ppool.tile([128, SL], F32, tag="proj", name=f"psv{s}_{c}")
                    for k in range(K2):
                        nc.tensor.matmul(ps_v[:], wv_t[k][:, :, cs], x_t[k][:], start=(k == 0), stop=(k == K2 - 1), perf_mode=DR)

                    sq = apool.tile([128, SL], BF16, tag=f"sq{c}", name=f"sq{s}_{c}")
                    nc.scalar.activation(sq[:], ps_q[:], SILU, bias=bq_t[c][:], scale=PS)
                    sk = apool.tile([128, SL], BF16, tag=f"sk{c}", name=f"sk{s}_{c}")
                    nc.scalar.activation(sk[:], ps_k[:], SILU, scale=PS)
                    vs = apool.tile([128, SL], BF16, tag=f"vs{c}", name=f"vs{s}_{c}")
                    nc.scalar.activation(vs[:], ps_v[:], IDENT, bias=bv_t[c][:], scale=PS)

                    # pkv = (silu_k + 1) * v
                    pk = apool.tile([128, SL], BF16, tag=f"pkv{c}", name=f"pkv{s}_{c}")
                    nc.vector.scalar_tensor_tensor(pk[:], sk[:], 1.0, vs[:], op0=ADD, op1=MULT)

                    # k_sum = cumsum(silu_k + 1), kv_sum = cumsum(pkv), chained across slabs
                    ks = spool.tile([128, SL], BF16, tag=f"ks{c}", name=f"ks{s}_{c}")
                    ik = 0.0 if s == 0 else prev_ks[c][:, prev_len - 1 : prev_len]
                    nc.vector.tensor_tensor_scan(ks[:], sk[:], ones_t[:, :SL], initial=ik, op0=ADD, op1=ADD)
                    kvs = spool.tile([128, SL], BF16, tag=f"kvs{c}", name=f"kvs{s}_{c}")
                    ikv = 0.0 if s == 0 else prev_kvs[c][:, prev_len - 1 : prev_len]
                    nc.vector.tensor_tensor_scan(kvs[:], pk[:], ones_t[:, :SL], initial=ikv, op0=ADD, op1=BYPASS)
                    prev_ks[c], prev_kvs[c] = ks, kvs

                    # phi_q = silu_q + 1 (4x tensor_scalar), pq_ks = phi_q * k_sum (2x TT)
                    phq = apool.tile([128, SL], BF16, tag=f"phq{c}", name=f"phq{s}_{c}")
                    nc.vector.tensor_scalar_add(phq[:], sq[:], 1.0)
                    pq = apool.tile([128, SL], BF16, tag=f"pq{c}", name=f"pq{s}_{c}")
                    nc.vector.tensor_mul(pq[:], phq[:], ks[:])

                    sq_l.append(phq), ks_l.append(ks), kvs_l.append(kvs), pq_l.append(pq)

                def tail_a(s, tt0, TL, sq_l, kvs_l, pq_l):
                    """den -> rec for slab s (emitted 1 slab late)."""
                    den_ps = dpool.tile([8, TL], F32, tag="den", name=f"den{s}")
                    for c in range(NCH):
                        nc.tensor.matmul(den_ps[:], em_t[c][:], pq_l[c][:], start=(c == 0), stop=(c == NCH - 1))
                    den_s = apool.tile([8, TL], F32, tag="dens", name=f"dens{s}")
                    nc.scalar.activation(den_s[:], den_ps[:], COPY, bias=1e-6)
                    rec32 = apool.tile([8, TL], F32, tag="rec32", name=f"rec32{s}")
                    nc.vector.reciprocal_approx_fast(rec32[:], den_s[:])
                    rec = apool.tile([8, TL], BF16, tag="rec", name=f"rec{s}")
                    nc.scalar.copy(rec[:], rec32[:])
                    return rec

                def tail_b(s, tt0, TL, sq_l, kvs_l, pq_l, rec):
                    """broadcast -> attn -> Wo for slab s (emitted 2 slabs late,
                    so rec is long since ready when the PE reaches the bcast)."""
                    tts = slice(tt0, tt0 + TL)
                    at_l = [apool.tile([128, 2, TL], FP8, tag=f"at{cc}", name=f"at{s}_{cc}") for cc in range(NCH // 2)]
                    for c in range(NCH):
                        cs = slice(128 * c, 128 * (c + 1))
                        rb = bpool.tile([128, TL], F32, tag="rb", name=f"rb{s}_{c}")
                        nc.tensor.matmul(rb[:], fm_t[:, cs], rec[:], start=True, stop=True)
                        nm = apool.tile([128, TL], BF16, tag=f"nm{c}", name=f"nm{s}_{c}")
                        nc.vector.tensor_mul(nm[:], sq_l[c][:], kvs_l[c][:])
                        nc.vector.tensor_mul(at_l[c // 2][:, c % 2, :], nm[:], rb[:])

                    for moo in range(MO // 2):
                        ot = outpool.tile([128, 2, TL], F32, tag=f"ot{moo}", name=f"ot{s}_{moo}")
                        for mo2 in range(2):
                            mo = 2 * moo + mo2
                            ms = slice(128 * mo, 128 * (mo + 1))
                            wo_ps = opool.tile([128, TL], F32, tag="wo", name=f"wo{s}_{mo}")
                            for kk in range(NCH // 2):
                                nc.tensor.matmul(wo_ps[:], wo_t[kk][:, :, ms], at_l[kk][:], start=(kk == 0), stop=(kk == NCH // 2 - 1), perf_mode=DR)
                            nc.scalar.activation(ot[:, mo2, :], wo_ps[:], COPY, scale=1.0 / 64.0)
                        nc.gpsimd.dma_start(
                            out_d[256 * moo : 256 * (moo + 1), tts].rearrange("(mo2 ki) t -> ki mo2 t", mo2=2),
                            ot[:])

                if prev_tail is not None:
                    rec_prev = tail_a(*prev_tail)
                    if prev_tail2 is not None:
                        tail_b(*prev_tail2)
                    prev_tail2 = (*prev_tail, rec_prev)
                prev_tail = (s, t0, SL, sq_l, kvs_l, pq_l)
                prev_len = SL

            rec_prev = tail_a(*prev_tail)
            tail_b(*prev_tail2)
            tail_b(*prev_tail, rec_prev)

    nc.compile()
    return nc


_NC_CACHE = {}


def _get_nc():
    if "nc" not in _NC_CACHE:
        _NC_CACHE["nc"] = build()
    return _NC_CACHE["nc"]


def make_in_maps(x, Wq, bq, Wk, Wv, bv, Wo, bo):
    bf = ml_dtypes.bfloat16
    f8 = ml_dtypes.float8_e4m3
    WS = 1.0 / (XS * PS)  # weight pre-scale so that psum * PS = W @ x exactly
    x3 = np.asarray(x, np.float32)[..., 0]                      # (B, C, T)
    E = np.zeros((CG, 8), np.float32)
    for ch in range(CG):
        E[ch, ch // DH] = 1.0
    ones = np.ones((128, S), bf)
    in_maps = []
    for core in range(8):
        b, g = core // 2, core % 2
        sl = slice(CG * g, CG * (g + 1))
        in_maps.append({
            "x": np.clip(x3[b] * XS, -240, 240).astype(f8),
            "wq": np.clip(np.ascontiguousarray(np.asarray(Wq, np.float32)[sl, :].T) * WS, -240, 240).astype(f8),
            "wk": np.clip(np.ascontiguousarray(np.asarray(Wk, np.float32)[sl, :].T) * WS, -240, 240).astype(f8),
            "wv": np.clip(np.ascontiguousarray(np.asarray(Wv, np.float32)[sl, :].T) * WS, -240, 240).astype(f8),
            "wo": np.clip(np.ascontiguousarray((np.asarray(Wo, np.float32)[:, sl] * 0.125).T) * 64.0, -240, 240).astype(f8),
            "bq": np.asarray(bq, np.float32)[sl].reshape(CG, 1).copy(),
            "bv": np.asarray(bv, np.float32)[sl].reshape(CG, 1).copy(),
            "emat": E.astype(bf),
            "fmat": np.ascontiguousarray(E.T).astype(bf),
            "ones": ones,
        })
    return in_maps


def assemble(results, bo):
    out = np.empty((B, C, T, 1), np.float32)
    bo_f = np.asarray(bo, np.float32)[:, None]
    for b in range(B):
        p0 = np.asarray(results[2 * b]["out"], np.float32)
        p1 = np.asarray(results[2 * b + 1]["out"], np.float32)
        out[b, :, :, 0] = p0 + p1 + bo_f
    return out


def kernel(x, Wq, bq, Wk, Wv, bv, Wo, bo):
    nc = _get_nc()
    in_maps = make_in_maps(x, Wq, bq, Wk, Wv, bv, Wo, bo)
    res = run_bass_kernel_spmd(nc, in_maps, core_ids=list(range(8)))
    return assemble(res.results, bo)
